# revision 1
# baseline (speedup 1.0000x reference)
"""Trainium2 Bass kernel for the AttnBlock-style attention module.

Reference computation (note softmax over axis=1, the *i* axis):
    q = wq @ x + bq ; k = wk @ x + bk ; v = wv @ x + bv      (per-pixel 1x1 conv)
    s[b,i,j] = (q[b,:,i] . k[b,:,j]) * C**-0.5
    attn = softmax_i(s)                                      (normalize over i!)
    out[b,c,i] = sum_j attn[b,i,j] v[b,c,j]
    y = wp @ out + bp

Sharding: 8 cores = 4 batches x 2 j-halves. The softmax over i is local to a
j-split (it normalizes each attention *column* j over all i). Each core gets x
with its j-half rotated to columns 0..2047 (a pure permutation of the pixel
axis, which passes through every per-pixel op and the i-softmax unchanged; the
host un-rotates the partial output). Each core:
  - computes q for all N=4096 pixels, k/v for columns 0..2047,
  - s_T[j, i] = k^T q   (j on partitions -> softmax reduction is free-axis),
  - attn = exp(s/16) stored unnormalized in bf16; per-j denominators D[j]
    from the fused activation accum_out; 1/D folded into v rows,
  - out_partial[c, i] = sum_{j in half} v_scaled[c,j] attn_T[j,i],
  - y_partial = wp @ out_partial   (bias bp added on host).
Host un-rotates and sums the two j-half partials per batch and adds bp.
"""

import numpy as np

import concourse.bass as bass
import concourse.mybir as mybir
import concourse.tile as tile
from concourse import bacc
from concourse import bass_utils

P = 128
B = 4
C = 256
N = 4096          # 64*64 pixels
NJ = 2048         # j columns per core
NJT = NJ // P     # 16 j tiles
SCALE = 1.0 / np.sqrt(C).item()   # 1/16

F32 = mybir.dt.float32
BF16 = mybir.dt.bfloat16
F32R = mybir.dt.float32r
AF = mybir.ActivationFunctionType


def _fr(ap):
    # fp32 data, float32r matmul mode: full PE rate when free dim >= 256.
    return ap.bitcast(F32R)


def _build_module():
    nc = bacc.Bacc("TRN2", target_bir_lowering=False, debug=False, num_devices=8)

    x_t = nc.dram_tensor("x", [C, N], BF16, kind="ExternalInput")
    w_t = nc.dram_tensor("wT", [3, C, C], BF16, kind="ExternalInput")  # wq.T, wk.T, (wp@wv).T
    b_t = nc.dram_tensor("b", [2, C], F32, kind="ExternalInput")      # bq, bk
    bv_t = nc.dram_tensor("bv", [1, C], F32, kind="ExternalInput")
    y_t = nc.dram_tensor("y", [C, N], F32, kind="ExternalOutput")

    with tile.TileContext(nc) as tc:
        _emit(nc, tc, x_t, w_t, b_t, bv_t, y_t)
    nc.compile()
    return nc


def _emit(nc, tc, x_t, w_t, b_t, bv_t, y_t):
    from contextlib import ExitStack

    with ExitStack() as top:
        const = top.enter_context(tc.tile_pool(name="const", bufs=1))
        big = top.enter_context(tc.tile_pool(name="big", bufs=1))

        # ---- constants (packed to dodge the 4KB alloc granularity) -----
        # w_all[:, 2*w + ci, :] = rows ci*128.. of weight w's transpose [ci, co]
        # slots 0..5: the three weights; slots 6,7: bv packed as f32 bits
        w_all = const.tile([P, 8, C], BF16, tag="w_all", name="w_all")
        # wk first (the k matmuls are the first consumers), then wq+w2
        nc.sync.dma_start(
            w_all[:, 2:4, :].rearrange("p w f -> p (w f)").rearrange(
                "p (c f) -> p c f", c=2),
            bass.AP(tensor=w_t, offset=C * C,
                    ap=[[C, P], [P * C, 2], [1, C]]),
        )
        nc.sync.dma_start(
            w_all[:, 0:2, :].rearrange("p w f -> p (w f)").rearrange(
                "p (c f) -> p c f", c=2),
            bass.AP(tensor=w_t, offset=0, ap=[[C, P], [P * C, 2], [1, C]]),
        )
        nc.sync.dma_start(
            w_all[:, 4:6, :].rearrange("p w f -> p (w f)").rearrange(
                "p (c f) -> p c f", c=2),
            bass.AP(tensor=w_t, offset=2 * C * C,
                    ap=[[C, P], [P * C, 2], [1, C]]),
        )

        def wslice(w, ci, ch):   # lhsT [128 ci, 128 co] for co half ch
            return w_all[:, 2 * w + ci, ch * P:(ch + 1) * P]

        # b_all columns: 0,1 = bq halves; 2,3 = bk halves
        b_all = const.tile([P, 4], F32, tag="b_all", name="b_all")
        # one DMA: b_all[p, 2*w+ch] = b[w, ch*128+p]
        nc.gpsimd.dma_start(
            b_all[:].rearrange("p (w c) -> p w c", c=2),
            bass.AP(tensor=b_t, offset=0, ap=[[1, P], [C, 2], [P, 2]]),
        )
        bv_sb = w_all[:, 6:8, :].rearrange("p a b -> p (a b)").bitcast(F32)
        nc.gpsimd.dma_start(
            bv_sb[:], bass.AP(tensor=bv_t, offset=0, ap=[[0, P], [1, C]])
        )

        # ---- persistent activations -----------------------------------
        q_bf = [big.tile([P, N], BF16, tag=f"q{ch}", name=f"q{ch}") for ch in range(2)]
        k_bf = [big.tile([P, NJ], BF16, tag=f"k{ch}", name=f"k{ch}") for ch in range(2)]
        v_all = big.tile([P, NJT, C], BF16, tag="v_all", name="v_all")
        attn = [big.tile([P, N], BF16, tag=f"a{jt}", name=f"a{jt}") for jt in range(NJT)]
        # d_all columns: 0:16 = per-jt sumexp, 32:48 = 1/D
        # cols 0:64 = per-(jt,iq) exp sums, 64:80 = D, 80:96 = 1/D
        d_all = big.tile([P, 96], F32, tag="d_all", name="d_all")
        dsum_all = d_all[:, 64:96]

        # ---- warmups: run while the x DMA streams in -------------------
        # ~8 dummy matmuls lift the PE HAM clock-gate to 8/8 before real
        # work arrives, and a dummy Exp pulls the ~2.7us ACT table load off
        # the critical path of the first score tile.
        with tc.tile_pool(name="warm", bufs=1) as wp_pool, \
             tc.tile_pool(name="warm_ps", bufs=1, space="PSUM") as wpp:
            wsb = wp_pool.tile([P, 512], BF16, tag="wsb", name="wsb")
            wex = wsb[:, 508:509]
            wps = wpp.tile([P, 512], F32, tag="wps", name="wps")
            nc.vector.memset(wsb[:], 0.0)
            for _ in range(6):
                nc.tensor.matmul(wps[:], wsb[:, 0:P], wsb[:],
                                 start=True, stop=True)
            nc.scalar.activation(wex[:], wps[:, 0:1], AF.Exp, scale=0.0)

        def bias_store(out_ap, ps, bias_ap, on_act):
            if on_act:
                nc.scalar.activation(out_ap, ps, AF.Identity, bias=bias_ap)
            else:
                nc.vector.tensor_scalar_add(out_ap, ps, bias_ap)

        psp = top.enter_context(tc.tile_pool(name="ps_s", bufs=2, space="PSUM"))

        def s_tile(jt, iq):
            # one [128,1024] score tile + exp(+accum) into the attn store
            ps = psp.tile([P, 1024], F32, tag="s", name="s_ps")
            for ch in range(2):
                lhs = k_bf[ch][:, jt * P:(jt + 1) * P]
                for t in range(2):
                    nc.tensor.matmul(
                        ps[:, t * 512:(t + 1) * 512], lhs,
                        q_bf[ch][:, iq * 1024 + t * 512: iq * 1024 + (t + 1) * 512],
                        start=(ch == 0), stop=(ch == 1),
                    )
            nc.scalar.activation(
                attn[jt][:, iq * 1024:(iq + 1) * 1024], ps[:],
                AF.Exp, scale=float(SCALE),
                accum_out=d_all[:, jt * 4 + iq: jt * 4 + iq + 1],
            )

        with tc.tile_pool(name="xload", bufs=1) as xp:
            # x arrives in [128, 1024] column blocks on alternating DMA
            # queues so the k/q matmuls start early.
            # One packed [P, 2, 1024] tile per x column-block, one DMA each:
            # separate tiles keep the dependency ranges clean (a single
            # packed [P, 2, N] tile made every read wait for the last block),
            # while keeping the dma_start count at 4 (more DMA instructions
            # measurably slowed the global matmul issue rate).
            XBLK = [(0, 512), (512, 512), (1024, 1024), (2048, 1024), (3072, 1024)]
            xb = [xp.tile([P, 2, w], BF16, tag=f"xb{b}", name=f"xb{b}")
                  for b, (lo, w) in enumerate(XBLK)]
            for b, (lo, w) in enumerate(XBLK):
                nc.sync.dma_start(
                    xb[b][:],
                    bass.AP(tensor=x_t, offset=lo,
                            ap=[[N, P], [P * N, 2], [1, w]]),
                )

            def xsl(ci, lo, size):
                # x[ci*128:(ci+1)*128, lo:lo+size] as an AP (within one block)
                for b, (blo, w) in enumerate(XBLK):
                    if blo <= lo and lo + size <= blo + w:
                        return xb[b][:, ci, lo - blo:lo - blo + size]
                raise AssertionError((lo, size))

            # ---- phase 1: k, q, vp projections, emitted block-wise -------
            # Work is ordered by which x column-block it needs, so the PE
            # starts as soon as block 0 lands and never waits for later
            # blocks (block spacing ~1.8us << ~8us of work per block).
            with tc.tile_pool(name="ps_qkv", bufs=4, space="PSUM") as pq:
                for blk in range(4):
                    if blk < 2:
                        # k chunks of this block (k covers columns 0..NJ)
                        for ch in range(2):
                            pss = [pq.tile([P, 512], F32, tag="ps", name="ps") for _ in range(2)]
                            for ci in range(2):
                                lhs = wslice(1, ci, ch)
                                for t2 in range(2):
                                    t = blk * 2 + t2
                                    nc.tensor.matmul(
                                        pss[t2][:], lhs,
                                        xsl(ci, t * 512, 512),
                                        start=(ci == 0), stop=(ci == 1),
                                    )
                            for t2 in range(2):
                                t = blk * 2 + t2
                                bias_store(k_bf[ch][:, t * 512:(t + 1) * 512], pss[t2][:],
                                           b_all[:, 2 + ch:3 + ch], on_act=(ch == 0))
                    # q chunks of this block
                    for ch in range(2):
                        pss = [pq.tile([P, 512], F32, tag="ps", name="ps") for _ in range(2)]
                        for ci in range(2):
                            lhs = wslice(0, ci, ch)
                            for t2 in range(2):
                                ic = blk * 2 + t2
                                nc.tensor.matmul(
                                    pss[t2][:], lhs,
                                    xsl(ci, ic * 512, 512),
                                    start=(ci == 0), stop=(ci == 1),
                                )
                        for t2 in range(2):
                            ic = blk * 2 + t2
                            bias_store(q_bf[ch][:, ic * 512:(ic + 1) * 512], pss[t2][:],
                                       b_all[:, ch:ch + 1], on_act=(ch == 0))
                    if blk == 1:
                        # k and the first 4 q chunks exist: start the jt0-3
                        # score tiles now so ACT's exp backlog begins while
                        # the PE finishes the projections (ACT is idle here)
                        for jt0 in range(4):
                            for iq0 in range(2):
                                s_tile(jt0, iq0)
                    elif blk == 2:
                        for jt0 in range(4):
                            s_tile(jt0, 2)
                    if blk < 2:
                        # vp_T[j, co] for this block's 8 j-tiles (wp folded
                        # into v on the host: W2 = wp@wv, b2 = wp@bv)
                        for jtg in range(2):
                            pss = [pq.tile([P, C], F32, tag="ps", name="ps") for _ in range(4)]
                            for ci in range(2):
                                for t in range(4):
                                    jt = blk * 8 + jtg * 4 + t
                                    nc.tensor.matmul(
                                        pss[t][:],
                                        xsl(ci, jt * P, P),
                                        w_all[:, 2 * 2 + ci, :],
                                        start=(ci == 0), stop=(ci == 1),
                                    )
                            for t in range(4):
                                nc.vector.tensor_add(
                                    v_all[:, blk * 8 + jtg * 4 + t, :], pss[t][:], bv_sb[:]
                                )

        # ---- phase 2+3 fused: scores/exp interleaved with y accum ------
        # s tiles are [128, 1024] (2 PSUM banks); the attention-weighted y
        # accumulation runs in four j-groups of 4 tiles, SBUF-accumulated.
        # Two accumulation chains of the previous group are emitted after
        # each score tile, so the PE's out-matmuls fill the stretches where
        # ACT is the bottleneck (exp).  All copies/adds go to DVE - the ACT
        # FIFO is saturated with exps and would stall the PSUM rotation.
        with tc.tile_pool(name="yaccp", bufs=1) as yp, \
             tc.tile_pool(name="ps_o", bufs=2, space="PSUM") as po, \
             tc.tile_pool(name="ysb", bufs=2) as ysb_pool:
            y_acc = yp.tile([P, 8, 1024], F32, tag="y_acc", name="y_acc")

            def out_chain(g, idx):
                # one accumulation chain: jts 4g..4g+3 into (iq, ch) slice
                iq, ch = divmod(idx, 2)
                ops = po.tile([P, 1024], F32, tag="og", name="og")
                for j2 in range(4 * g, 4 * g + 4):
                    lhs = v_all[:, j2, ch * P:(ch + 1) * P]
                    for t in range(2):
                        nc.tensor.matmul(
                            ops[:, t * 512:(t + 1) * 512], lhs,
                            attn[j2][:, iq * 1024 + t * 512: iq * 1024 + (t + 1) * 512],
                            start=(j2 == 4 * g), stop=(j2 == 4 * g + 3),
                        )
                if g == 0:
                    nc.vector.tensor_copy(y_acc[:, idx, :], ops[:])
                elif g < 3:
                    nc.vector.tensor_add(y_acc[:, idx, :], ops[:], y_acc[:, idx, :])
                else:
                    y_sb = ysb_pool.tile([P, 1024], F32, tag="ysb", name="ysb")
                    nc.vector.tensor_add(y_sb[:], ops[:], y_acc[:, idx, :])
                    nc.sync.dma_start(
                        y_t.ap()[ch * P:(ch + 1) * P, iq * 1024:(iq + 1) * 1024],
                        y_sb[:],
                    )

            for jt in range(NJT):
                for iq in range(4):
                    if jt < 4 and iq < 3:
                        continue  # pre-emitted during the qkv phase
                    s_tile(jt, iq)
                # per-jt denominator (sum the 4 chunk sums) + vp scaling
                nc.vector.reduce_sum(
                    dsum_all[:, jt:jt + 1], d_all[:, jt * 4:jt * 4 + 4],
                    axis=mybir.AxisListType.X,
                )
                nc.vector.reciprocal(
                    dsum_all[:, 16 + jt:17 + jt], dsum_all[:, jt:jt + 1]
                )
                nc.vector.tensor_scalar_mul(
                    v_all[:, jt, :], v_all[:, jt, :],
                    dsum_all[:, 16 + jt:17 + jt],
                )
                if jt >= 4:
                    g = jt // 4 - 1
                    off = (jt % 4) * 2
                    out_chain(g, off)
                    out_chain(g, off + 1)
            for idx in range(8):
                out_chain(3, idx)

_nc_cache = None
LAST_EXEC_TIME_NS = None


def _get_nc():
    global _nc_cache
    if _nc_cache is None:
        _nc_cache = _build_module()
    return _nc_cache


def kernel(x, wq, bq, wk, bk, wv, bv, wp, bp):
    global LAST_EXEC_TIME_NS
    nc = _get_nc()

    import ml_dtypes
    bf = ml_dtypes.bfloat16
    x = np.asarray(x, dtype=np.float32).reshape(B, C, N).astype(bf)
    wq32 = np.asarray(wq, dtype=np.float32)
    wk32 = np.asarray(wk, dtype=np.float32)
    wv32 = np.asarray(wv, dtype=np.float32)
    wp32 = np.asarray(wp, dtype=np.float32)
    w2 = wp32 @ wv32                      # fold the output projection into v
    wT = np.ascontiguousarray(np.stack([wq32.T, wk32.T, w2.T])).astype(bf)
    b2 = np.ascontiguousarray(np.stack([
        np.asarray(bq, dtype=np.float32), np.asarray(bk, dtype=np.float32)
    ]))
    bv2 = np.ascontiguousarray((wp32 @ np.asarray(bv, dtype=np.float32)).reshape(1, C))
    bp1 = np.asarray(bp, dtype=np.float32).reshape(C)

    in_maps = []
    for core in range(8):
        b, h = divmod(core, 2)
        xb = x[b] if h == 0 else np.ascontiguousarray(np.roll(x[b], -NJ, axis=1))
        in_maps.append({"x": xb, "wT": wT, "b": b2, "bv": bv2})

    res = bass_utils.run_bass_kernel_spmd(nc, in_maps, core_ids=list(range(8)))
    if res.exec_time_ns is not None:
        LAST_EXEC_TIME_NS = res.exec_time_ns

    y = np.zeros((B, C, N), np.float32)
    for b in range(B):
        y[b] = res.results[2 * b]["y"] + np.roll(res.results[2 * b + 1]["y"], NJ, axis=1)
    y += bp1.reshape(1, C, 1)
    return y.reshape(B, C, 64, 64)



# revision 2
# speedup vs baseline: 1.1621x; 1.1621x over previous
"""Trainium2 Bass kernel for the AttnBlock-style attention module.

Reference computation (note softmax over axis=1, the *i* axis):
    q = wq @ x + bq ; k = wk @ x + bk ; v = wv @ x + bv      (per-pixel 1x1 conv)
    s[b,i,j] = (q[b,:,i] . k[b,:,j]) * C**-0.5
    attn = softmax_i(s)                                      (normalize over i!)
    out[b,c,i] = sum_j attn[b,i,j] v[b,c,j]
    y = wp @ out + bp

Sharding: 8 cores = 4 batches x 2 j-halves. The softmax over i is local to a
j-split (it normalizes each attention *column* j over all i). Each core gets x
with its j-half rotated to columns 0..2047 (a pure permutation of the pixel
axis, which passes through every per-pixel op and the i-softmax unchanged; the
host un-rotates the partial output). Each core:
  - computes q for all N=4096 pixels, k/v for columns 0..2047 (bf16),
  - s_T[j, i] = k^T q   (j on partitions -> softmax reduction is free-axis),
  - attn'' = exp(s/16 - CA) stored unnormalized in fp8e4m3 (CA keeps the max
    under the e4m3 ceiling of 240); per-j denominators D'' from the fused
    activation accum_out,
  - v8 = v_pre / D''_j in fp8e4m3, where v_pre = PRE*(wp@wv) x + PRE*(wp@bv)
    is host-prescaled by PRE = 256*e^CA so v8 sits in e4m3's normal range,
  - out_partial[c, i] = sum_{j in half} v8[c,j] attn''[j,i] via fp8 DoubleRow
    matmuls (2x PE rate; the e^-CA in attn'' cancels against the e^+CA inside
    1/D'', so out_partial = PRE * true partial).
Host un-rotates, sums the two j-half partials per batch, divides by PRE and
adds bp.
"""

import numpy as np

import concourse.bass as bass
import concourse.mybir as mybir
import concourse.tile as tile
from concourse import bacc
from concourse import bass_utils

P = 128
B = 4
C = 256
N = 4096          # 64*64 pixels
NJ = 2048         # j columns per core
NJT = NJ // P     # 16 j tiles
SCALE = 1.0 / np.sqrt(C).item()   # 1/16
CA = 3.0                          # exp bias: attn'' = exp(s/16 - CA)
PRE = 256.0 * float(np.exp(CA))   # host prescale on the folded v projection

F32 = mybir.dt.float32
BF16 = mybir.dt.bfloat16
F8E4 = mybir.dt.float8e4
AF = mybir.ActivationFunctionType
DR = mybir.MatmulPerfMode.DoubleRow


def _build_module():
    nc = bacc.Bacc("TRN2", target_bir_lowering=False, debug=False, num_devices=8)

    x_t = nc.dram_tensor("x", [C, N], BF16, kind="ExternalInput")
    w_t = nc.dram_tensor("wT", [3, C, C], BF16, kind="ExternalInput")  # wq.T, wk.T, (PRE*wp@wv).T
    b_t = nc.dram_tensor("b", [2, C], F32, kind="ExternalInput")      # bq, bk
    bv_t = nc.dram_tensor("bv", [1, C], F32, kind="ExternalInput")    # PRE*(wp@bv)
    y_t = nc.dram_tensor("y", [C, N], F32, kind="ExternalOutput")

    with tile.TileContext(nc) as tc:
        _emit(nc, tc, x_t, w_t, b_t, bv_t, y_t)
    nc.compile()
    return nc


def _chunks(jt):
    # i-chunks per j-tile: phase-1 pre-emitted jts use 4x1024, the rest use
    # 1536/1536/1024 (fewer, larger exp calls amortize ACT's ~350cyc/call).
    if jt < 4:
        return [(0, 1024), (1024, 1024), (2048, 1024), (3072, 1024)]
    return [(0, 1536), (1536, 1536), (3072, 1024)]


def _emit(nc, tc, x_t, w_t, b_t, bv_t, y_t):
    from contextlib import ExitStack

    with ExitStack() as top:
        const = top.enter_context(tc.tile_pool(name="const", bufs=1))
        big = top.enter_context(tc.tile_pool(name="big", bufs=1))

        # ---- constants (packed to dodge the 4KB alloc granularity) -----
        # w_all[:, 2*w + ci, :] = rows ci*128.. of weight w's transpose [ci, co]
        # slots 0..5: the three weights; slots 6,7: bv packed as f32 bits
        w_all = const.tile([P, 8, C], BF16, tag="w_all", name="w_all")
        # wk first (the k matmuls are the first consumers) on the sync queue;
        # wq / w2 go on other engine queues so they don't delay the x blocks.
        nc.sync.dma_start(
            w_all[:, 2:4, :].rearrange("p w f -> p (w f)").rearrange(
                "p (c f) -> p c f", c=2),
            bass.AP(tensor=w_t, offset=C * C,
                    ap=[[C, P], [P * C, 2], [1, C]]),
        )
        nc.vector.dma_start(
            w_all[:, 0:2, :].rearrange("p w f -> p (w f)").rearrange(
                "p (c f) -> p c f", c=2),
            bass.AP(tensor=w_t, offset=0, ap=[[C, P], [P * C, 2], [1, C]]),
        )
        nc.scalar.dma_start(
            w_all[:, 4:6, :].rearrange("p w f -> p (w f)").rearrange(
                "p (c f) -> p c f", c=2),
            bass.AP(tensor=w_t, offset=2 * C * C,
                    ap=[[C, P], [P * C, 2], [1, C]]),
        )

        def wslice(w, ci, ch):   # lhsT [128 ci, 128 co] for co half ch
            return w_all[:, 2 * w + ci, ch * P:(ch + 1) * P]

        # b_all columns: 0,1 = bq halves; 2,3 = bk halves
        b_all = const.tile([P, 4], F32, tag="b_all", name="b_all")
        # one DMA: b_all[p, 2*w+ch] = b[w, ch*128+p]
        nc.gpsimd.dma_start(
            b_all[:].rearrange("p (w c) -> p w c", c=2),
            bass.AP(tensor=b_t, offset=0, ap=[[1, P], [C, 2], [P, 2]]),
        )
        bv_sb = w_all[:, 6:8, :].rearrange("p a b -> p (a b)").bitcast(F32)
        nc.gpsimd.dma_start(
            bv_sb[:], bass.AP(tensor=bv_t, offset=0, ap=[[0, P], [1, C]])
        )

        # ---- persistent activations -----------------------------------
        q_bf = [big.tile([P, N], BF16, tag=f"q{ch}", name=f"q{ch}") for ch in range(2)]
        k_bf = [big.tile([P, NJ], BF16, tag=f"k{ch}", name=f"k{ch}") for ch in range(2)]
        v_all = big.tile([P, NJT, C], BF16, tag="v_all", name="v_all")
        v8 = big.tile([P, NJT, C], F8E4, tag="v8", name="v8")
        # attn'' paired by jt for DoubleRow: attn2[t][:, s, :] = jt 2t+s
        attn2 = [big.tile([P, 2, N], F8E4, tag=f"a{t}", name=f"a{t}")
                 for t in range(NJT // 2)]
        # d_all columns: 0:64 = per-(jt,chunk) exp sums (4 slots per jt,
        # unused slots stay zero), 64:80 = D'', 80:96 = 1/D''
        d_all = big.tile([P, 96], F32, tag="d_all", name="d_all")
        dsum_all = d_all[:, 64:96]
        nc.vector.memset(d_all[:], 0.0)

        # ---- warmups: run while the x DMA streams in -------------------
        # ~6 dummy matmuls lift the PE HAM clock-gate to 8/8 before real
        # work arrives, and a dummy Exp pulls the ~2.7us ACT table load off
        # the critical path of the first score tile.
        with tc.tile_pool(name="warm", bufs=1) as wp_pool, \
             tc.tile_pool(name="warm_ps", bufs=1, space="PSUM") as wpp:
            wsb = wp_pool.tile([P, 512], BF16, tag="wsb", name="wsb")
            wex = wsb[:, 508:509]
            wps = wpp.tile([P, 512], F32, tag="wps", name="wps")
            nc.vector.memset(wsb[:], 0.0)
            for _ in range(6):
                nc.tensor.matmul(wps[:], wsb[:, 0:P], wsb[:],
                                 start=True, stop=True)
            nc.scalar.activation(wex[:], wps[:, 0:1], AF.Exp, scale=0.0)

        def bias_store(out_ap, ps, bias_ap, on_act):
            if on_act:
                nc.scalar.activation(out_ap, ps, AF.Identity, bias=bias_ap)
            else:
                nc.vector.tensor_scalar_add(out_ap, ps, bias_ap)

        def s_chunk(pool, width, jt, c3, lo, w):
            # one [128, w] score chunk + exp into the fp8 attn store
            ps = pool.tile([P, width], F32, tag="s", name="s_ps")
            for ch in range(2):
                lhs = k_bf[ch][:, jt * P:(jt + 1) * P]
                for t in range(w // 512):
                    nc.tensor.matmul(
                        ps[:, t * 512:(t + 1) * 512], lhs,
                        q_bf[ch][:, lo + t * 512: lo + (t + 1) * 512],
                        start=(ch == 0), stop=(ch == 1),
                    )
            nc.scalar.activation(
                attn2[jt // 2][:, jt % 2, lo:lo + w], ps[:, :w],
                AF.Exp, scale=float(SCALE), bias=float(-CA),
                accum_out=d_all[:, jt * 4 + c3: jt * 4 + c3 + 1],
            )

        with tc.tile_pool(name="xload", bufs=1) as xp, \
             tc.tile_pool(name="ps_s1", bufs=2, space="PSUM") as ps1:
            # x arrives in [128, ...] column blocks; one packed [P, 2, w]
            # tile per block, one DMA each, all on the sync queue right
            # behind the wk weights.
            XBLK = [(0, 512), (512, 512), (1024, 1024), (2048, 1024), (3072, 1024)]
            xb = [xp.tile([P, 2, w], BF16, tag=f"xb{b}", name=f"xb{b}")
                  for b, (lo, w) in enumerate(XBLK)]
            for b, (lo, w) in enumerate(XBLK):
                nc.sync.dma_start(
                    xb[b][:],
                    bass.AP(tensor=x_t, offset=lo,
                            ap=[[N, P], [P * N, 2], [1, w]]),
                )

            def xsl(ci, lo, size):
                # x[ci*128:(ci+1)*128, lo:lo+size] as an AP (within one block)
                for b, (blo, w) in enumerate(XBLK):
                    if blo <= lo and lo + size <= blo + w:
                        return xb[b][:, ci, lo - blo:lo - blo + size]
                raise AssertionError((lo, size))

            # ---- phase 1: k, q, vp projections, emitted block-wise -------
            # Work is ordered by which x column-block it needs, so the PE
            # starts as soon as block 0 lands and never waits for later
            # blocks.
            with tc.tile_pool(name="ps_qkv", bufs=4, space="PSUM") as pq:
                for blk in range(4):
                    if blk < 2:
                        # k chunks of this block (k covers columns 0..NJ)
                        for ch in range(2):
                            pss = [pq.tile([P, 512], F32, tag="ps", name="ps") for _ in range(2)]
                            for ci in range(2):
                                lhs = wslice(1, ci, ch)
                                for t2 in range(2):
                                    t = blk * 2 + t2
                                    nc.tensor.matmul(
                                        pss[t2][:], lhs,
                                        xsl(ci, t * 512, 512),
                                        start=(ci == 0), stop=(ci == 1),
                                    )
                            for t2 in range(2):
                                t = blk * 2 + t2
                                bias_store(k_bf[ch][:, t * 512:(t + 1) * 512], pss[t2][:],
                                           b_all[:, 2 + ch:3 + ch], on_act=(ch == 0))
                    # q chunks of this block
                    for ch in range(2):
                        pss = [pq.tile([P, 512], F32, tag="ps", name="ps") for _ in range(2)]
                        for ci in range(2):
                            lhs = wslice(0, ci, ch)
                            for t2 in range(2):
                                ic = blk * 2 + t2
                                nc.tensor.matmul(
                                    pss[t2][:], lhs,
                                    xsl(ci, ic * 512, 512),
                                    start=(ci == 0), stop=(ci == 1),
                                )
                        for t2 in range(2):
                            ic = blk * 2 + t2
                            bias_store(q_bf[ch][:, ic * 512:(ic + 1) * 512], pss[t2][:],
                                       b_all[:, ch:ch + 1], on_act=False)
                    if blk == 1:
                        # k and the first 4 q chunks exist: start the jt0-3
                        # score tiles now so ACT's exp backlog begins while
                        # the PE finishes the projections (ACT is idle here)
                        for jt0 in range(4):
                            for c3 in range(2):
                                s_chunk(ps1, 1024, jt0, c3, c3 * 1024, 1024)
                    elif blk == 2:
                        for jt0 in range(4):
                            s_chunk(ps1, 1024, jt0, 2, 2048, 1024)
                    if blk < 2:
                        # vp_T[j, co] for this block's 8 j-tiles (wp folded
                        # into v on the host: W2 = PRE*wp@wv, b2 = PRE*wp@bv);
                        # the bias add runs on the otherwise-idle gpsimd.
                        for jtg in range(2):
                            pss = [pq.tile([P, C], F32, tag="ps", name="ps") for _ in range(4)]
                            for ci in range(2):
                                for t in range(4):
                                    jt = blk * 8 + jtg * 4 + t
                                    nc.tensor.matmul(
                                        pss[t][:],
                                        xsl(ci, jt * P, P),
                                        w_all[:, 2 * 2 + ci, :],
                                        start=(ci == 0), stop=(ci == 1),
                                    )
                            for t in range(4):
                                nc.gpsimd.tensor_add(
                                    v_all[:, blk * 8 + jtg * 4 + t, :], pss[t][:], bv_sb[:]
                                )

        # ---- phase 2+3 fused: scores/exp interleaved with y accum ------
        # s chunks are up to [128, 1536] (3 PSUM banks, 2-buf rotation);
        # the attention-weighted y accumulation runs as fp8 DoubleRow
        # matmuls in four j-groups, SBUF-accumulated in [128,512] chunks.
        with tc.tile_pool(name="yaccp", bufs=1) as yp, \
             tc.tile_pool(name="ps_s2", bufs=2, space="PSUM") as ps2, \
             tc.tile_pool(name="ps_o", bufs=2, space="PSUM") as po, \
             tc.tile_pool(name="ysb", bufs=2) as ysb_pool:
            y_acc = yp.tile([P, 16, 512], F32, tag="y_acc", name="y_acc")

            def out_chain(g, idx):
                # one accumulation chain: jts 4g..4g+3 (attn2 tiles 2g,2g+1)
                # into (iq512, ch) slice idx, via fp8 DoubleRow
                iq, ch = divmod(idx, 2)
                ops = po.tile([P, 512], F32, tag="og", name="og")
                for t in (2 * g, 2 * g + 1):
                    nc.tensor.matmul(
                        ops[:], v8[:, 2 * t:2 * t + 2, ch * P:(ch + 1) * P],
                        attn2[t][:, :, iq * 512:(iq + 1) * 512],
                        start=(t == 2 * g), stop=(t == 2 * g + 1),
                        perf_mode=DR,
                    )
                if g == 0:
                    nc.gpsimd.tensor_copy(y_acc[:, idx, :], ops[:])
                elif g < 3:
                    nc.vector.tensor_add(y_acc[:, idx, :], ops[:], y_acc[:, idx, :])
                else:
                    y_sb = ysb_pool.tile([P, 512], F32, tag="ysb", name="ysb")
                    nc.vector.tensor_add(y_sb[:], ops[:], y_acc[:, idx, :])
                    nc.sync.dma_start(
                        y_t.ap()[ch * P:(ch + 1) * P, iq * 512:(iq + 1) * 512],
                        y_sb[:],
                    )

            for jt in range(NJT):
                for c3, (lo, w) in enumerate(_chunks(jt)):
                    if jt < 4 and c3 < 3:
                        continue  # pre-emitted during the qkv phase
                    s_chunk(ps2, 1536, jt, c3, lo, w)
                # per-jt denominator (sum the chunk sums; unused slots are 0)
                nc.vector.reduce_sum(
                    dsum_all[:, jt:jt + 1], d_all[:, jt * 4:jt * 4 + 4],
                    axis=mybir.AxisListType.X,
                )
                nc.vector.reciprocal(
                    dsum_all[:, 16 + jt:17 + jt], dsum_all[:, jt:jt + 1]
                )
                nc.vector.tensor_scalar_mul(
                    v8[:, jt, :], v_all[:, jt, :],
                    dsum_all[:, 16 + jt:17 + jt],
                )
                if jt >= 4:
                    g = jt // 4 - 1
                    off = (jt % 4) * 4
                    for m in range(4):
                        out_chain(g, off + m)
            for idx in range(16):
                out_chain(3, idx)

_nc_cache = None
LAST_EXEC_TIME_NS = None


def _get_nc():
    global _nc_cache
    if _nc_cache is None:
        _nc_cache = _build_module()
    return _nc_cache


def kernel(x, wq, bq, wk, bk, wv, bv, wp, bp):
    global LAST_EXEC_TIME_NS
    nc = _get_nc()

    import ml_dtypes
    bf = ml_dtypes.bfloat16
    x = np.asarray(x, dtype=np.float32).reshape(B, C, N).astype(bf)
    wq32 = np.asarray(wq, dtype=np.float32)
    wk32 = np.asarray(wk, dtype=np.float32)
    wv32 = np.asarray(wv, dtype=np.float32)
    wp32 = np.asarray(wp, dtype=np.float32)
    w2 = (wp32 @ wv32) * PRE              # fold output projection + prescale
    wT = np.ascontiguousarray(np.stack([wq32.T, wk32.T, w2.T])).astype(bf)
    b2 = np.ascontiguousarray(np.stack([
        np.asarray(bq, dtype=np.float32), np.asarray(bk, dtype=np.float32)
    ]))
    bv2 = np.ascontiguousarray(
        (PRE * (wp32 @ np.asarray(bv, dtype=np.float32))).reshape(1, C))
    bp1 = np.asarray(bp, dtype=np.float32).reshape(C)

    in_maps = []
    for core in range(8):
        b, h = divmod(core, 2)
        xb = x[b] if h == 0 else np.ascontiguousarray(np.roll(x[b], -NJ, axis=1))
        in_maps.append({"x": xb, "wT": wT, "b": b2, "bv": bv2})

    res = bass_utils.run_bass_kernel_spmd(nc, in_maps, core_ids=list(range(8)))
    if res.exec_time_ns is not None:
        LAST_EXEC_TIME_NS = res.exec_time_ns

    y = np.zeros((B, C, N), np.float32)
    inv = 1.0 / PRE
    for b in range(B):
        y[b] = (res.results[2 * b]["y"]
                + np.roll(res.results[2 * b + 1]["y"], NJ, axis=1)) * inv
    y += bp1.reshape(1, C, 1)
    return y.reshape(B, C, 64, 64)
